# revision 1
# baseline (speedup 1.0000x reference)
"""Trainium2 Bass kernel: Conv3d(3->24, k=3, valid) + bias -> min over depth -> softmax over channels.

Full inputs: x (128, 3, 16, 64, 64) f32, conv_weight (24, 3, 3, 3, 3), conv_bias (24,).
Output: (128, 24, 62, 62) f32. Data-parallel over 8 cores (16 batch each).

Per-core design (PE-bound, ~12us per h-block of 5 output rows, 13 blocks):
  Conv = 2 accumulating matmul passes per (h-block, depth) instead of 3:
    K = 126 partitions = 2 w-shift copies x (3 kd x 7 rows x 3 cin).
    Pass 1 covers kw in {0,1} in one matmul (shift-0 half + host-prepared
    shift-1 dup); pass 2 covers kw=2 (zero weights on the shift-0 half,
    shift-1 half read at +1 column). M = 120 = 5 h-outputs x 24 channels;
    N = 496 = 8 batch x 62 w, two batch halves at PSUM columns 0 and 512 of
    a [120, 1024] 2-bank tile (496-wide halves must not cross the 2KB bank
    boundary). 1984 PE rows per (block, depth) at 0.4167 ns/row.
  Input fp16, streamed as 6 DMAs per h-block (one per shift x kd sub-block:
  21 partitions x 14 depth slots x 2KB), raw 3-level access patterns with
  the depth-slot dim striding the same source axis as kd; double-buffered.
  Min over 14 depths: gpsimd has no PSUM port or ALU ops, so drains split
  between Act (9 psum->fp16 copies) and DVE (5 fused psum mins); DVE also
  min-folds the Act tiles in fp16 2x mode. All epilogue ops use (2, 496)
  strided views that skip the PSUM pad columns.
  Softmax: exp on Act (conv bias folded into the activation bias), block-
  diag ones-matmul for per-pixel denominator sums, denominators extracted
  (Act) and stored; the final broadcast divide rides the host gather pass
  (gpsimd cannot min/mult and Act+DVE are saturated by the min tree).
  exp / denominator / store work for older blocks is emitted at fixed
  depth hooks inside later conv blocks so no engine head-blocks the psum
  ring. Output stored fp16 as (h, o, b, w) + (h, b, w) sums; host
  transposes, upcasts, divides.
"""

import os as _os_mod
import numpy as np

import concourse.bacc as bacc


def _env(k, d):
    return _os_mod.environ.get(k, d)

import concourse.bass as bass
import concourse.mybir as mybir
import concourse.tile as tile
from concourse.ap import AP
from concourse.bass_utils import run_bass_kernel_spmd

F32 = mybir.dt.float32
FP16 = mybir.dt.float16

B_CORE = 16
C_IN = 3
D_IN = 16
H_IN = 64
W_IN = 64
O = 24
D_OUT = 14
HW_OUT = 62
HB = [0, 5, 10, 15, 20, 25, 30, 35, 40, 45, 50, 55, 57]
NB = len(HB)

# dram xd layout [u, d, h, c, b, w] (fp16), strides in elements
S_C = B_CORE * W_IN          # 1024
S_H = C_IN * S_C             # 3072
S_D = H_IN * S_H             # 196608
S_U = D_IN * S_D             # 3145728

_CACHE = {}


def build_host_tensors(conv_weight, conv_bias):
    """lhsT [126, 240]: partition p = u*63 + kd*21 + j*3 + c.
    cols 0:120 = pass1 (u=0 -> kw0 taps, u=1 -> kw1), cols 120:240 = pass2
    (u=0 zero, u=1 -> kw2). col m = hp*24 + o, nonzero iff 0 <= j-hp <= 2."""
    L = np.zeros((128, 240), np.float32)
    for u in range(2):
        for kd in range(3):
            for c in range(C_IN):
                for j in range(7):
                    p = u * 63 + kd * 21 + j * 3 + c
                    for hp in range(5):
                        kh = j - hp
                        if 0 <= kh <= 2:
                            # pass1
                            L[p, hp * 24:hp * 24 + O] = conv_weight[:, c, kd, kh, u]
                            # pass2: only u=1 half carries kw=2
                            if u == 1:
                                L[p, 120 + hp * 24:120 + hp * 24 + O] = \
                                    conv_weight[:, c, kd, kh, 2]
    ones = np.zeros((120, 120), np.float32)
    for hp in range(5):
        ones[hp * 24:(hp + 1) * 24, hp * 24:(hp + 1) * 24] = 1.0
    biasv = np.zeros((128, 1), np.float32)
    for hp in range(5):
        biasv[hp * 24:(hp + 1) * 24, 0] = conv_bias
    return L.astype(np.float16), ones.astype(np.float16), biasv


def build_bass():
    nc = bacc.Bacc(None, target_bir_lowering=False)
    xd = nc.dram_tensor("xd", [2, D_IN, H_IN, C_IN, B_CORE, W_IN], FP16,
                        kind="ExternalInput")
    lw = nc.dram_tensor("lw", [128, 240], FP16, kind="ExternalInput")
    ones = nc.dram_tensor("ones", [120, 120], FP16, kind="ExternalInput")
    biasv = nc.dram_tensor("biasv", [128, 1], F32, kind="ExternalInput")
    y = nc.dram_tensor("y", [HW_OUT, O, B_CORE, HW_OUT], FP16,
                       kind="ExternalOutput")
    yd = nc.dram_tensor("yd", [HW_OUT, B_CORE, HW_OUT], FP16,
                        kind="ExternalOutput")
    xd_h = xd[:, :, :, :, :, :].tensor
    y_h = y[:, :, :, :].tensor
    yd_h = yd[:, :, :].tensor

    MIN = mybir.AluOpType.min
    MULT = mybir.AluOpType.mult

    with tile.TileContext(nc) as tc:
        with (
            tc.tile_pool(name="const", bufs=1) as constp,
            tc.tile_pool(name="xs", bufs=int(_env("XTBUFS", "2"))) as xsp,
            tc.tile_pool(name="accs", bufs=int(_env("ACCB", "3"))) as accp,
            tc.tile_pool(name="acts", bufs=int(_env("ACTB", "2"))) as actp,
            tc.tile_pool(name="mp", bufs=int(_env("MPB", "3"))) as mp,
            tc.tile_pool(name="etp", bufs=int(_env("ETB", "4"))) as etp,
            tc.tile_pool(name="op", bufs=int(_env("OPB", "3"))) as outp,
            tc.tile_pool(name="ps", bufs=4, space="PSUM") as psp,
        ):
            lwt = constp.tile([128, 240], FP16, tag="lw")
            onest = constp.tile([128, 120], FP16, tag="ones")
            biast = constp.tile([128, 1], F32, tag="bias")

            st = {}  # per-block saved tiles

            def load_consts():
                nc.sync.dma_start(lwt[:, :], lw[:, :])
                nc.sync.dma_start(biast[:, :], biasv[:, :])
                nc.sync.dma_start(onest[0:120, :], ones[:, :])

            def load(i, split=0):
                h0 = HB[i]
                xt = xsp.tile([128, D_OUT, 1024], FP16, tag="xt", name="xt")
                # DMA APs are limited to 3 dims; one DMA per (u, kd) block of
                # 21 partitions, (j, c) pre-merged (contiguous), depth slot
                # striding the same source axis as kd. For the first block,
                # split the slot range so dt=0 compute can start sooner.
                slot_ranges = [(0, 4), (4, D_OUT)] if split else [(0, D_OUT)]
                for s0, s1 in slot_ranges:
                    for u in range(2):
                        for kd in range(3):
                            p0 = u * 63 + kd * 21
                            src = AP(xd_h,
                                     u * S_U + (kd + s0) * S_D + h0 * S_H,
                                     [[S_C, 21], [S_D, s1 - s0], [1, 1024]])
                            eng = nc.gpsimd if (split and u == 1) else nc.sync
                            eng.dma_start(xt[p0:p0 + 21, s0:s1, :], src)
                st[i] = {"xt": xt}

            # per-depth drain assignment, interleaved so consecutive psum-ring
            # slots are freed by different engines (Act copy keeps pace only
            # every other slot; gpsimd has no PSUM port so Pool folds the
            # fp16 Act tiles instead).
            # drain assignment: gpsimd/Pool has no usable elementwise ALU op
            # on TRN2 (TensorTensor fails the Pool ISA check in codegen), so
            # psum drains split across Act (copies) and DVE (fused mins), and
            # DVE min-folds the Act tiles in fp16 2x mode.
            DVE_DTS = [int(c) for c in _env("DVEDTS", "0,3,7,10,12").split(",")]
            ACT_DTS = [d for d in range(D_OUT) if d not in DVE_DTS]
            def V(t, p=120):
                return t[0:p, :].rearrange("p (h f) -> p h f", h=2)[:, :, 0:496]

            def conv_block(i, hooks=None):
                s = st[i]
                xt = s["xt"]
                accD = accp.tile([128, 1024], FP16, tag="accD", name="accD")
                accA = accp.tile([128, 1024], FP16, tag="accA", name="accA")
                m = mp.tile([128, 1024], FP16, tag="m", name="m")
                acts = {}
                nact = 0
                for dt in range(D_OUT):
                    ps = psp.tile([128, 1024], F32, tag="ps", name="ps")

                    v = xt[0:126, dt:dt + 1, :].rearrange(
                        "p s (b w) -> p (s b) w", b=B_CORE)
                    for half in range(2):
                        c0 = half * 512
                        b0 = half * 8
                        nc.tensor.matmul(
                            ps[0:120, c0:c0 + 496], lwt[0:126, 0:120],
                            v[:, b0:b0 + 8, 0:HW_OUT],
                            start=True, stop=False)
                        nc.tensor.matmul(
                            ps[0:120, c0:c0 + 496], lwt[0:126, 120:240],
                            v[:, b0:b0 + 8, 1:1 + HW_OUT],
                            start=False, stop=True)
                    if dt in ACT_DTS:
                        a = actp.tile([128, 1024], FP16, tag=f"t{dt}",
                                      name=f"t{dt}")
                        nc.scalar.copy(V(a), V(ps))
                        acts[dt] = a
                        nact += 1
                        if nact == 2:
                            ks = sorted(acts)
                            nc.vector.tensor_tensor(
                                V(accA), V(acts[ks[0]]), V(acts[ks[1]]), MIN)
                        elif nact > 2 and dt != ACT_DTS[-1]:
                            nc.vector.tensor_tensor(
                                V(accA), V(acts[dt]), V(accA), MIN)
                        elif dt == ACT_DTS[-1]:
                            if ACT_DTS[-1] > DVE_DTS[-1]:
                                # accA already merged accD: fused final
                                nc.vector.tensor_tensor(
                                    V(m), V(acts[dt]), V(accA), MIN)
                            else:
                                nc.vector.tensor_tensor(
                                    V(accA), V(acts[dt]), V(accA), MIN)
                    elif dt == DVE_DTS[0]:
                        nc.vector.tensor_scalar_min(
                            V(accD), V(ps), 60000.0)
                    else:
                        nc.vector.tensor_tensor(
                            V(accD), V(ps), V(accD), MIN)
                    if dt == DVE_DTS[-1]:
                        # merge the DVE psum-chain; if DVE drains end last,
                        # this merge IS the final and writes m
                        if DVE_DTS[-1] > ACT_DTS[-1]:
                            nc.vector.tensor_tensor(
                                V(m), V(accD), V(accA), MIN)
                        else:
                            nc.vector.tensor_tensor(
                                V(accA), V(accD), V(accA), MIN)
                    if hooks and dt in hooks:
                        for f in hooks[dt]:
                            f()
                s["m"] = m

            def exp_block(i):
                s = st[i]
                et = etp.tile([128, 1024], FP16, tag="et", name="et")
                nc.scalar.activation(V(et), V(s["m"]),
                                     mybir.ActivationFunctionType.Exp,
                                     bias=biast[0:120, 0:1], scale=1.0)
                s["et"] = et

            def denom_block(i):
                s = st[i]
                dps = psp.tile([128, 1024], F32, tag="ps", name="dps")

                et = s["et"]
                for half in range(2):
                    c0 = half * 512
                    nc.tensor.matmul(dps[0:120, c0:c0 + 496],
                                     onest[0:120, 0:120],
                                     et[0:120, c0:c0 + 496],
                                     start=True, stop=True)
                s["dps"] = dps

            def recip_block(i):
                # extract the denominator sums from psum (Act copy); the
                # final broadcast divide happens on the host gather pass.
                s = st[i]
                dx = outp.tile([128, 1024], FP16, tag="dx", name="dx")
                if _env("DXENG", "act") == "act":
                    nc.scalar.copy(V(dx), V(s["dps"]))
                else:
                    nc.vector.tensor_scalar_min(V(dx), V(s["dps"]), 6.0e4)
                s["dx"] = dx

            def finish_block(i):
                s = st.pop(i)
                h0 = HB[i]
                dst = AP(y_h, h0 * O * B_CORE * HW_OUT,
                         [[O * B_CORE * HW_OUT, 5], [B_CORE * HW_OUT, O],
                          [496, 2], [1, 496]])
                nc.sync.dma_start(dst, s["et"][0:120, :].rearrange(
                    "p (h f) -> p h f", h=2)[:, :, 0:496])
                ddst = AP(yd_h, h0 * B_CORE * HW_OUT,
                          [[B_CORE * HW_OUT, 5], [496, 2], [1, 496]])
                nc.sync.dma_start(ddst, s["dx"][0:120, :].rearrange(
                    "(g o) (h f) -> g o h f", o=24, h=2)[:, 0:1, :, 0:496])

            def denom_extract(j):
                denom_block(j)
                recip_block(j)

            load_consts()
            load(0, split=1)
            for i in range(NB + 2):
                if i + 1 < NB:
                    load(i + 1)
                hooks = {}
                H_EXP = int(_env("HEXP", "6"))
                H_DEN = int(_env("HDEN", "8"))
                H_FIN = int(_env("HFIN", "10"))
                if 0 <= i - 1 < NB:
                    hooks.setdefault(H_EXP, []).append(
                        lambda j=i - 1: exp_block(j))
                if 0 <= i - 2 < NB:
                    hooks.setdefault(H_DEN, []).append(
                        lambda j=i - 2: denom_extract(j))
                    hooks.setdefault(H_FIN, []).append(
                        lambda j=i - 2: finish_block(j))
                if i < NB:
                    conv_block(i, hooks=hooks)
                else:
                    for dt in sorted(hooks):
                        for f in hooks[dt]:
                            f()
    nc.finalize()
    return nc


def kernel(x, conv_weight, conv_bias):
    x = np.asarray(x, dtype=np.float32)
    conv_weight = np.asarray(conv_weight, dtype=np.float32)
    conv_bias = np.asarray(conv_bias, dtype=np.float32)
    L, ones, biasv = build_host_tensors(conv_weight, conv_bias)
    if "nc" not in _CACHE:
        _CACHE["nc"] = build_bass()
    nc = _CACHE["nc"]
    core_ids = list(range(8))
    # (b, c, d, h, w) -> (d, h, c, b, w)
    x_t = np.transpose(x, (2, 3, 1, 0, 4)).astype(np.float16)
    in_maps = []
    for i in core_ids:
        xc = x_t[:, :, :, i * B_CORE:(i + 1) * B_CORE, :]
        xd = np.zeros((2,) + xc.shape, np.float16)
        xd[0] = xc
        xd[1, :, :, :, :, :-1] = xc[:, :, :, :, 1:]
        in_maps.append({"xd": xd, "lw": L, "ones": ones, "biasv": biasv})
    res = run_bass_kernel_spmd(nc, in_maps, core_ids)
    parts = []
    for i in core_ids:
        et = np.transpose(res.results[i]["y"], (2, 1, 0, 3)).astype(np.float32)
        d = np.transpose(res.results[i]["yd"], (1, 0, 2)).astype(np.float32)
        parts.append(et / d[:, None, :, :])
    return np.ascontiguousarray(np.concatenate(parts, axis=0))


if __name__ == "__main__":
    rng = np.random.default_rng(0)
    x = rng.standard_normal((128, 3, 16, 64, 64), dtype=np.float32)
    w = (rng.standard_normal((24, 3, 3, 3, 3)) * 0.1).astype(np.float32)
    b = (rng.standard_normal(24) * 0.1).astype(np.float32)
    out = kernel(x=x, conv_weight=w, conv_bias=b)
    print("out", out.shape, out.dtype)

